# revision 1
# baseline (speedup 1.0000x reference)
"""Trainium2 Bass kernel for nn_MixerGroupedTiedAttention.

Sharding: 8 cores = (batch B=2) x (kv-group G=4). Each core handles one
batch element and one group of 4 q-heads + their shared kv-head:
  - qkv / gate projections: tensor-parallel column slices of W_qkv / W_g
  - k_rope (head-tied) replicated (folded into the per-core W slab)
  - sliding-window attention (W=1024) computed block-sparse over 128x128
    token tiles: per query-tile only the 9 valid key-tiles are computed.

Numerics: projections and score matmuls run in float32r (full-rate fp32
mode, ~1.5e-4 component error measured on HW); exp probabilities and V
run in bf16 with fp32 PSUM accumulation. Softmax skips max-subtraction:
logits are bounded (|s| < ~45) so exp stays comfortably in fp32 range.

Device program (per core):
  Phase A (per 128-token tile): x rows -> PE-transpose -> xT slab;
    fused qkv|krope and gate projections (tokens on PSUM partitions);
    RMSNorm via ACT Square+accum_out, Sqrt, DVE reciprocal; RoPE on DVE;
    per-token scale rows fused via tensor_scalar; q/k re-transposed to
    (d, t) via PE; v kept token-major with a fused ones column (gives
    the softmax denominator for free); silu gate.
  Phase B (per head, rolling over token tiles): scores s^T = kT.T @ qT
    -> (j, i) PSUM, Exp on ACT, boundary-block masks, u bf16 (kept for a
    9-tile sliding window); y|l = u.T @ [v|1] accumulated over key
    tiles; normalize by 1/l, gate, DMA out.
"""

import numpy as np
import ml_dtypes

D_MODEL = 2048
N_HEADS = 16
N_KV = 4
D_HEAD = 128
D1 = 64
D2 = 64
WSIZE = 1024
EPS = 1e-6
ROPE_BASE = 10000.0
B = 2
T = 2048
NCORES = 8
HPC = 4  # q heads per core
NT = T // 128  # 16 token tiles
NWB = WSIZE // 128 + 1  # 9 key tiles per query tile
PCOLS = 768  # q(512) | kv(128) | krope(64) | zero pad(64)

_BF16 = ml_dtypes.bfloat16
_built = {}


def _build_nc():
    """Build the single-core SPMD Bass program (same program all 8 cores)."""
    if "nc" in _built:
        return _built["nc"]
    import concourse.bacc as bacc
    import concourse.tile as tile
    from concourse import mybir

    # All ACT functions this kernel uses (Copy/Square/Ln/Exp) live in the
    # "natural_log_exp_and_others" table set. The table-load pass greedily
    # picks the first set containing each function, which alternates table
    # loads (~2.7us each) between sets; restrict every other set's
    # advertised membership so exactly one table set is ever loaded.
    if not getattr(bacc, "_act_tables_pinned", False):
        _orig_gat = bacc.get_activation_tables
        _mine = {
            mybir.ActivationFunctionType.Copy,
            mybir.ActivationFunctionType.Identity,
            mybir.ActivationFunctionType.Square,
            mybir.ActivationFunctionType.Ln,
            mybir.ActivationFunctionType.Exp,
        }

        def _pinned_gat(arch):
            tabs = _orig_gat(arch)
            return {
                name: (funcs if name == "natural_log_exp_and_others"
                       else funcs - _mine)
                for name, funcs in tabs.items()
            }

        bacc.get_activation_tables = _pinned_gat
        bacc._act_tables_pinned = True

    f32 = mybir.dt.float32
    f32r = mybir.dt.float32r
    bf16 = mybir.dt.bfloat16
    AF = mybir.ActivationFunctionType
    OP = mybir.AluOpType

    nc = bacc.Bacc("TRN2", target_bir_lowering=False, debug=False)

    def din(name, shape, dt):
        return nc.dram_tensor(name, shape, dt, kind="ExternalInput").ap()

    xb = din("xb", [T, D_MODEL], f32r)
    wqkv = din("wqkv", [D_MODEL, PCOLS], f32r)
    wg = din("wg", [D_MODEL, 512], f32r)
    brk = din("brk", [128, 64], f32)
    # host pre-swizzled to partition-major: [128, NT, d]
    cost = din("cost", [128, NT, 32], f32)
    sint = din("sint", [128, NT, 32], f32)
    crow = din("crow", [128, NT, HPC], f32)
    mdiag = din("mdiag", [128, 128], bf16)
    mfar = din("mfar", [128, 128], bf16)
    ident = din("ident", [128, 128], f32r)
    out = nc.dram_tensor("out", [T, 512], f32, kind="ExternalOutput").ap()

    with tile.TileContext(nc) as tc:
        # ---- persistent SBUF ----
        with tc.tile_pool(name="persist", bufs=1) as pp:
            # per-k weight tiles -> DMA deps stay per-chunk, so the first
            # projection matmuls don't wait for the whole weight load
            wqkv_sb = [pp.tile([128, PCOLS], f32r, name=f"wqkv_{k}")
                       for k in range(16)]
            wg_sb = [pp.tile([128, 512], f32r, name=f"wg_{k}")
                     for k in range(16)]
            brk_sb = pp.tile([128, 64], f32)
            cos_sb = pp.tile([128, NT, 32], f32)
            sin_sb = pp.tile([128, NT, 32], f32)
            crow_sb = pp.tile([128, NT, HPC], f32)
            mdiag_sb = pp.tile([128, 128], bf16)
            mfar_sb = pp.tile([128, 128], bf16)
            ident_sb = pp.tile([128, 128], f32r)
            eps_sb = pp.tile([128, 1], f32)
            nc.vector.memset(eps_sb[:], EPS)
            qT_sb = pp.tile([128, HPC, T], f32r)
            kT_sb = pp.tile([128, T], f32r)
            vaug_sb = pp.tile([128, NT, 132], bf16)
            gs_sb = pp.tile([128, NT, 512], bf16)

            # constants + first x tiles first, then weights per-k so the
            # pipeline head isn't gated on the full 10MB weight load
            nc.sync.dma_start(ident_sb[:], ident[:])
            nc.sync.dma_start(brk_sb[:], brk[:])
            nc.sync.dma_start(cos_sb[:], cost[:])
            nc.sync.dma_start(sin_sb[:], sint[:])
            nc.sync.dma_start(crow_sb[:], crow[:])
            nc.sync.dma_start(mdiag_sb[:], mdiag[:])
            nc.sync.dma_start(mfar_sb[:], mfar[:])
            # ones column of v_aug (softmax-denominator accumulator)
            nc.vector.memset(vaug_sb[:, :, 128], 1.0)

            # ================= Phase A: projections =================
            with tc.tile_pool(name="xrows", bufs=3) as xrp, \
                 tc.tile_pool(name="xT", bufs=2) as xtp, \
                 tc.tile_pool(name="ptr", bufs=1, space="PSUM") as ptr, \
                 tc.tile_pool(name="pproj", bufs=2, space="PSUM") as ppj, \
                 tc.tile_pool(name="pwg", bufs=1, space="PSUM") as pwg, \
                 tc.tile_pool(name="ptq", bufs=1, space="PSUM") as ptq, \
                 tc.tile_pool(name="awork", bufs=2) as awp:
                # x-tile DMAs interleaved with the weight stream so the
                # transpose pipeline has work during the weight cold start
                xpre = {}
                for ti in range(2):
                    xs = xrp.tile([128, D_MODEL], f32r, name=f"x_{ti}", tag="x")
                    nc.sync.dma_start(xs[:], xb[ti * 128:(ti + 1) * 128, :])
                    xpre[ti] = xs
                for k in range(16):
                    nc.sync.dma_start(wqkv_sb[k][:],
                                      wqkv[k * 128:(k + 1) * 128, :])
                    nc.sync.dma_start(wg_sb[k][:], wg[k * 128:(k + 1) * 128, :])
                    if k % 2 == 0 and 2 + k // 2 < NT:
                        ti = 2 + k // 2
                        xs = xrp.tile([128, D_MODEL], f32r, name=f"x_{ti}",
                                      tag="x")
                        nc.sync.dma_start(
                            xs[:], xb[ti * 128:(ti + 1) * 128, :])
                        xpre[ti] = xs

                def emit_transposes(ti):
                    x_slab = xpre.pop(ti, None)
                    if x_slab is None:
                        x_slab = xrp.tile([128, D_MODEL], f32r,
                                          name=f"x_{ti}", tag="x")
                        nc.sync.dma_start(
                            x_slab[:], xb[ti * 128:(ti + 1) * 128, :])
                    xT = xtp.tile([128, 16, 128], f32r, name=f"xT_{ti}",
                                  tag="xT")
                    for kq in range(4):  # 4 transposes per psum bank, 1 evac
                        tr = ptr.tile([128, 512], f32r, name=f"tr_{ti}_{kq}",
                                      tag="tr")
                        for j in range(4):
                            k = kq * 4 + j
                            nc.tensor.transpose(
                                tr[:, j * 128:(j + 1) * 128],
                                x_slab[:, k * 128:(k + 1) * 128],
                                ident_sb[:],
                            )
                        nc.scalar.copy(
                            xT[:, kq * 4:(kq + 1) * 4, :],
                            tr[:].rearrange("p (k t) -> p k t", k=4))
                    return xT

                xTq = {0: emit_transposes(0)}
                for ti in range(NT):
                    tsl = slice(ti * 128, (ti + 1) * 128)
                    xT = xTq.pop(ti)
                    if ti + 1 < NT:
                        xTq[ti + 1] = emit_transposes(ti + 1)

                    # projections: tokens on PSUM partitions.
                    # one accumulation group per bank region, regions >=256
                    # wide so f32r runs at full rate.
                    proj = ppj.tile([128, PCOLS], f32, name=f"proj_{ti}",
                                    tag="proj")
                    gps = pwg.tile([128, 512], f32, name=f"gps_{ti}", tag="gps")
                    for dst, getsrc in (
                        (proj[:, 0:512], lambda k: wqkv_sb[k][:, 0:512]),
                        (proj[:, 512:768], lambda k: wqkv_sb[k][:, 512:768]),
                        (gps[:], lambda k: wg_sb[k][:]),
                    ):
                        for k in range(16):
                            nc.tensor.matmul(dst, xT[:, k, :], getsrc(k),
                                             start=(k == 0), stop=(k == 15))

                    # rmsnorm scales: sumsq over each 128-wide head chunk
                    ss = awp.tile([128, 5], f32, name=f"ss_{ti}", tag="ss")
                    sq = awp.tile([128, 128], f32, name=f"sq_{ti}", tag="sq")
                    for hc in range(5):
                        nc.scalar.activation(sq[:], proj[:, hc * 128:(hc + 1) * 128],
                                             AF.Square, accum_out=ss[:, hc:hc + 1])
                    # r = rsqrt(mean+eps) via exp(-0.5*ln(.)) — keeps every
                    # ACT function in the ln/exp table set (no table reloads)
                    lnm = awp.tile([128, 5], f32, name=f"lnm_{ti}", tag="lnm")
                    nc.scalar.activation(lnm[:], ss[:], AF.Ln,
                                         scale=1.0 / 128.0, bias=eps_sb[:])
                    r = awp.tile([128, 5], f32, name=f"r_{ti}", tag="r")
                    nc.scalar.activation(r[:], lnm[:], AF.Exp, scale=-0.5)
                    rc = awp.tile([128, HPC], f32, name=f"rc_{ti}", tag="rc")
                    nc.vector.tensor_mul(rc[:], r[:, 0:4], crow_sb[:, ti, :])

                    # q heads: rope batched across all 4 heads via step-0
                    # broadcast of cos/sin, per-head scale, transpose to (d,t)
                    cos_t = cos_sb[:, ti, :]
                    sin_t = sin_sb[:, ti, :]
                    cosb = cos_t[:, None, :].broadcast_to([128, 4, 32])
                    sinb = sin_t[:, None, :].broadcast_to([128, 4, 32])
                    qh = proj[:, 0:512].rearrange("p (h d) -> p h d", h=4)
                    x1 = qh[:, :, 64:96]
                    x2 = qh[:, :, 96:128]
                    t1 = awp.tile([128, 4, 32], f32, name=f"t1_{ti}", tag="t1")
                    t2 = awp.tile([128, 4, 32], f32, name=f"t2_{ti}", tag="t2")
                    rp = awp.tile([128, 4, 64], f32, name=f"rp_{ti}", tag="rp")
                    nc.vector.tensor_mul(t1[:], x1, cosb)
                    nc.vector.tensor_mul(t2[:], x2, sinb)
                    nc.vector.tensor_add(rp[:, :, 0:32], t1[:], t2[:])
                    nc.vector.tensor_mul(t1[:], x2, cosb)
                    nc.vector.tensor_mul(t2[:], x1, sinb)
                    nc.vector.tensor_sub(rp[:, :, 32:64], t1[:], t2[:])
                    qtps = ptq.tile([128, 512], f32r, name=f"qtps_{ti}",
                                    tag="qtps")
                    for h in range(HPC):
                        off = h * 128
                        qf = awp.tile([128, 128], f32r, name=f"qf_{ti}_{h}",
                                      tag="qf")
                        nc.vector.tensor_scalar(
                            qf[:, 0:64], proj[:, off:off + 64],
                            rc[:, h:h + 1], None, OP.mult)
                        nc.gpsimd.tensor_scalar(
                            qf[:, 64:128], rp[:, h, :], rc[:, h:h + 1], None,
                            OP.mult)
                        nc.tensor.transpose(
                            qtps[:, off:off + 128], qf[:], ident_sb[:])
                    nc.scalar.copy(
                        qT_sb[:, 0:4, tsl],
                        qtps[:, 0:512].rearrange("p (h t) -> p h t", h=4))

                    # kv head -> v (token-major) and k tied half
                    nc.vector.tensor_scalar(
                        vaug_sb[:, ti, 0:128], proj[:, 512:640],
                        r[:, 4:5], None, OP.mult)
                    kpre = awp.tile([128, 128], f32r, name=f"kpre_{ti}", tag="kpre")
                    nc.vector.tensor_scalar(
                        kpre[:, 0:64], proj[:, 512:576],
                        r[:, 4:5], None, OP.mult)
                    # k_rope: bias, rope (no norm)
                    krf = awp.tile([128, 64], f32, name=f"krf_{ti}", tag="krf")
                    nc.vector.tensor_add(krf[:], proj[:, 640:704], brk_sb[:])
                    t1 = awp.tile([128, 32], f32, name=f"kt1_{ti}", tag="t1")
                    t2 = awp.tile([128, 32], f32, name=f"kt2_{ti}", tag="t2")
                    nc.vector.tensor_mul(t1[:], krf[:, 0:32], cos_t)
                    nc.vector.tensor_mul(t2[:], krf[:, 32:64], sin_t)
                    nc.vector.tensor_add(kpre[:, 64:96], t1[:], t2[:])
                    nc.vector.tensor_mul(t1[:], krf[:, 32:64], cos_t)
                    nc.vector.tensor_mul(t2[:], krf[:, 0:32], sin_t)
                    nc.vector.tensor_sub(kpre[:, 96:128], t1[:], t2[:])
                    ktps = ptq.tile([128, 128], f32r, name=f"ktps_{ti}", tag="ktps")
                    nc.tensor.transpose(ktps[:], kpre[:], ident_sb[:])
                    nc.scalar.copy(kT_sb[:, tsl], ktps[:])

                    # gate: silu = g / (1 + exp(-g)) — exp keeps the single
                    # ACT table set; +1 on gpsimd; reciprocal+mul on DVE
                    gsg = awp.tile([128, 512], f32, name=f"gsg_{ti}", tag="gsg")
                    nc.scalar.activation(gsg[:], gps[:], AF.Exp, scale=-1.0)
                    gw = awp.tile([128, 512], f32, name=f"gw_{ti}", tag="gw")
                    nc.gpsimd.tensor_scalar_add(gw[:], gsg[:], 1.0)
                    gwi = awp.tile([128, 512], f32, name=f"gwi_{ti}", tag="gwi")
                    nc.vector.reciprocal(gwi[:], gw[:])
                    nc.vector.tensor_mul(gs_sb[:, ti, :], gps[:], gwi[:])


            # ================= Phase B: attention =================
            with tc.tile_pool(name="u", bufs=11) as up, \
                 tc.tile_pool(name="psS", bufs=2, space="PSUM") as psS, \
                 tc.tile_pool(name="psY", bufs=2, space="PSUM") as psY, \
                 tc.tile_pool(name="bwork", bufs=4) as bwp:
                for h in range(HPC):
                    uslabs = {}
                    for t in range(NT + 1):
                        # ---- scores for key tile tj = t ----
                        if t < NT:
                            tj = t
                            nblk = min(NWB, NT - tj)
                            ni = nblk * 128
                            s_ps = psS.tile([128, 1152], f32,
                                            name=f"s_{h}_{tj}", tag="s")
                            u_tj = up.tile([128, 1152], bf16,
                                           name=f"u_{h}_{tj}", tag="u")
                            uslabs[tj] = u_tj
                            i0 = tj * 128
                            for c0 in range(0, ni, 512):
                                c1 = min(c0 + 512, ni)
                                nc.tensor.matmul(
                                    s_ps[:, c0:c1],
                                    kT_sb[:, tj * 128:(tj + 1) * 128],
                                    qT_sb[:, h, i0 + c0:i0 + c1],
                                    start=True, stop=True)
                            nc.scalar.activation(
                                u_tj[:, 0:ni], s_ps[:, 0:ni], AF.Exp)
                            nc.gpsimd.tensor_mul(
                                u_tj[:, 0:128], u_tj[:, 0:128], mdiag_sb[:])
                            if nblk == NWB:
                                nc.gpsimd.tensor_mul(
                                    u_tj[:, 1024:1152],
                                    u_tj[:, 1024:1152], mfar_sb[:])
                        # ---- output for query tile ti = t - 1 (one step
                        # behind S so Y never waits on the slab exp'd this
                        # step) ----
                        ti = t - 1
                        if ti < 0:
                            continue
                        nblk = min(ti, NWB - 1) + 1
                        y_ps = psY.tile([128, 132], f32, name=f"y_{h}_{ti}",
                                        tag="y")
                        for w, tj2 in enumerate(range(ti - nblk + 1, ti + 1)):
                            woff = (ti - tj2) * 128
                            nc.tensor.matmul(
                                y_ps[:, 0:129],
                                uslabs[tj2][:, woff:woff + 128],
                                vaug_sb[:, tj2, 0:129],
                                start=(w == 0), stop=(w == nblk - 1))
                        linv = bwp.tile([128, 1], f32, name=f"li_{h}_{ti}",
                                        tag="li")
                        nc.vector.reciprocal(linv[:], y_ps[:, 128:129])
                        outt = bwp.tile([128, 128], f32, name=f"o_{h}_{ti}",
                                        tag="o")
                        nc.vector.scalar_tensor_tensor(
                            outt[:], y_ps[:, 0:128], linv[:],
                            gs_sb[:, ti, h * 128:(h + 1) * 128],
                            OP.mult, OP.mult)
                        nc.sync.dma_start(
                            out[ti * 128:(ti + 1) * 128, h * 128:(h + 1) * 128],
                            outt[:])

    nc.compile()
    _built["nc"] = nc
    return nc


def _host_inputs(hidden_states, W_qkv, W_rk, b_rk, softmax_scaler, W_g):
    """Per-core input dicts (host-side sharding / constant prep)."""
    inv_freq = 1.0 / (ROPE_BASE ** (np.arange(0, D2, 2, dtype=np.float32) / D2))
    tpos = np.arange(T, dtype=np.float32)
    freqs = tpos[:, None] * inv_freq[None, :]
    cost = np.cos(freqs).astype(np.float32)
    sint = np.sin(freqs).astype(np.float32)
    logpos = np.log(np.minimum(tpos + 1.0, float(WSIZE))).astype(np.float32)
    scale = logpos / np.float32(np.sqrt(D_HEAD))

    ii = np.arange(128)
    mdiag = (ii[:, None] <= ii[None, :]).astype(_BF16)  # (j, i): j <= i
    mfar = (ii[:, None] >= ii[None, :]).astype(_BF16)   # (j, i): j >= i
    ident = np.eye(128, dtype=np.float32)
    brk_t = np.broadcast_to(
        np.asarray(b_rk, np.float32)[None, :], (128, 64)).copy()

    xf = np.asarray(hidden_states, np.float32)
    wqkv_f = np.asarray(W_qkv, np.float32)
    wrk_f = np.asarray(W_rk, np.float32)
    wg_f = np.asarray(W_g, np.float32)
    scaler = np.asarray(softmax_scaler, np.float32)
    zpad = np.zeros((D_MODEL, 64), np.float32)

    in_maps = []
    for c in range(NCORES):
        b, g = c // N_KV, c % N_KV
        qcols = wqkv_f[:, 4 * g * 128:(4 * g + 4) * 128]
        kvcols = wqkv_f[:, (N_HEADS + g) * 128:(N_HEADS + g + 1) * 128]
        crow = scale[:, None] * scaler[None, 4 * g:4 * g + 4]
        in_maps.append({
            "xb": np.ascontiguousarray(xf[b]),
            "wqkv": np.ascontiguousarray(
                np.concatenate([qcols, kvcols, wrk_f, zpad], axis=1)),
            "wg": np.ascontiguousarray(
                wg_f[:, 4 * g * 128:(4 * g + 4) * 128]),
            "brk": brk_t,
            # pre-swizzle (T, d) -> (128, NT, d) partition-major
            "cost": np.ascontiguousarray(
                cost.reshape(NT, 128, 32).transpose(1, 0, 2)),
            "sint": np.ascontiguousarray(
                sint.reshape(NT, 128, 32).transpose(1, 0, 2)),
            "crow": np.ascontiguousarray(
                crow.reshape(NT, 128, HPC).transpose(1, 0, 2)).astype(
                    np.float32),
            "mdiag": mdiag,
            "mfar": mfar,
            "ident": ident,
        })
    return in_maps


def kernel(hidden_states, W_qkv, W_rk, b_rk, softmax_scaler, W_g):
    from concourse.bass_utils import run_bass_kernel_spmd

    nc = _build_nc()
    in_maps = _host_inputs(hidden_states, W_qkv, W_rk, b_rk,
                           softmax_scaler, W_g)
    res = run_bass_kernel_spmd(nc, in_maps, list(range(NCORES)))
    outf = np.empty((B, T, N_HEADS, D_HEAD), np.float32)
    for c in range(NCORES):
        b, g = c // N_KV, c % N_KV
        outf[b, :, 4 * g:4 * g + 4, :] = res.results[c]["out"].reshape(
            T, HPC, D_HEAD)
    return outf



# revision 35
# speedup vs baseline: 1.0993x; 1.0993x over previous
"""Trainium2 Bass kernel for nn_MixerGroupedTiedAttention.

Sharding: 8 cores = (batch B=2) x (kv-group G=4). Each core handles one
batch element and one group of 4 q-heads + their shared kv-head:
  - qkv / gate projections: tensor-parallel column slices of W_qkv / W_g
  - k_rope (head-tied) replicated (folded into the per-core W slab)
  - sliding-window attention (W=1024) computed block-sparse over 128x128
    token tiles.

Single fused pipeline (v2): projections (PE-heavy, ACT-light) and
attention (ACT-heavy, PE-light) are interleaved per query-tile PAIR so
the tensor engine never drains while exp slabs run. Attention is
organized per QUERY tile (not per key tile as v1), so tile t's scores
only need data from tiles <= t and the whole kernel is one software
pipeline. x is transposed on the host (the device never runs the 256
PE transposes / PSUM evacuations v1 spent ~60us on).

Numerics: projections and score matmuls in float32r (full-rate fp32);
exp probabilities and V in bf16 with fp32 PSUM accumulation (measured
~2.5e-3 rel err headroom vs the 2e-2 gate; bf16 scores measured 2.2e-2
on CPU and are NOT safe). Score matmuls keep f32r at full rate by
moving a 256-wide PAIR of query tiles per matmul (f32r needs >=256
moving columns for 1 cycle/row); the pair's 10-block union window has
2 invalid corner half-blocks that are simply never read by the y
matmuls.

PSUM budget (8 banks x 2KB):
  pa pool   2 bufs x [128,768] f32 = 4 banks  (qkv proj; gate reuses
            [0:448] after evacuation; 64 gate cols ride in the qkv slab
            at [704:768] so both accumulation regions are >=256 wide)
  s tile    [128,1156] f32 = 3 banks (scores in 4-block passes over
            banks 0-1; y accumulator [1024:1153] alone in bank 2 --
            concurrent PSUM accumulation groups must not share a bank)
  tq pool   [128,384] f32r = 1 bank (q/k PE transposes, two waves)
"""

import numpy as np
import ml_dtypes

D_MODEL = 2048
N_HEADS = 16
N_KV = 4
D_HEAD = 128
D1 = 64
D2 = 64
WSIZE = 1024
EPS = 1e-6
ROPE_BASE = 10000.0
B = 2
T = 2048
NCORES = 8
HPC = 4  # q heads per core
NT = T // 128  # 16 token tiles
NPAIR = NT // 2  # 8 query-tile pairs

_BF16 = ml_dtypes.bfloat16
_built = {}


def _build_nc():
    """Build the single-core SPMD Bass program (same program all 8 cores)."""
    if "nc" in _built:
        return _built["nc"]
    import concourse.bacc as bacc
    import concourse.tile as tile
    from concourse import mybir

    # All ACT functions this kernel uses (Copy/Square/Ln/Exp) live in the
    # "natural_log_exp_and_others" table set. The table-load pass greedily
    # picks the first set containing each function, which alternates table
    # loads (~2.7us each) between sets; restrict every other set's
    # advertised membership so exactly one table set is ever loaded.
    if not getattr(bacc, "_act_tables_pinned", False):
        _orig_gat = bacc.get_activation_tables
        _mine = {
            mybir.ActivationFunctionType.Copy,
            mybir.ActivationFunctionType.Identity,
            mybir.ActivationFunctionType.Square,
            mybir.ActivationFunctionType.Ln,
            mybir.ActivationFunctionType.Exp,
        }

        def _pinned_gat(arch):
            tabs = _orig_gat(arch)
            return {
                name: (funcs if name == "natural_log_exp_and_others"
                       else funcs - _mine)
                for name, funcs in tabs.items()
            }

        bacc.get_activation_tables = _pinned_gat
        bacc._act_tables_pinned = True

    f32 = mybir.dt.float32
    f32r = mybir.dt.float32r
    bf16 = mybir.dt.bfloat16
    AF = mybir.ActivationFunctionType
    OP = mybir.AluOpType

    nc = bacc.Bacc("TRN2", target_bir_lowering=False, debug=False)

    def din(name, shape, dt):
        return nc.dram_tensor(name, shape, dt, kind="ExternalInput").ap()

    # host pre-transposed + swizzled x: [128(p=d%128), NT, 16(k=d//128), 128(t)]
    xt = din("xt", [128, NT, 16, 128], f32r)
    # [2048, 768] = [q 512 | kv 128 | krope 64 | wg cols 448:512]
    wqkv = din("wqkv", [D_MODEL, 768], f32r)
    wg = din("wg", [D_MODEL, 448], f32r)
    brk = din("brk", [128, 64], f32)
    # host pre-swizzled to partition-major: [128, NT, d]
    cost = din("cost", [128, NT, 32], f32)
    sint = din("sint", [128, NT, 32], f32)
    crow = din("crow", [128, NT, HPC], f32)
    mdiag = din("mdiag", [128, 128], bf16)
    mfar = din("mfar", [128, 128], bf16)
    ident = din("ident", [128, 128], f32r)
    out = nc.dram_tensor("out", [T, 512], f32, kind="ExternalOutput").ap()

    with tile.TileContext(nc) as tc:
        with tc.tile_pool(name="persist", bufs=1) as pp:
            # per-k weight tiles -> DMA deps stay per-chunk, so the first
            # tiles' projection matmuls start as soon as each chunk lands
            wqkv_sb = [pp.tile([128, 768], f32r, name=f"wqkv_{k}")
                       for k in range(16)]
            wg_sb = [pp.tile([128, 448], f32r, name=f"wg_{k}")
                     for k in range(16)]
            brk_sb = pp.tile([128, 64], f32)
            cos_sb = pp.tile([128, NT, 32], f32)
            sin_sb = pp.tile([128, NT, 32], f32)
            crow_sb = pp.tile([128, NT, HPC], f32)
            mdiag_sb = pp.tile([128, 128], bf16)
            mfar_sb = pp.tile([128, 128], bf16)
            ident_sb = pp.tile([128, 128], f32r)
            eps_sb = pp.tile([128, 1], f32)
            nc.vector.memset(eps_sb[:], EPS)
            kT_sb = pp.tile([128, T], f32r)
            vaug_sb = pp.tile([128, NT, 132], bf16)

            nc.sync.dma_start(ident_sb[:], ident[:])
            nc.sync.dma_start(brk_sb[:], brk[:])
            nc.sync.dma_start(cos_sb[:], cost[:])
            nc.sync.dma_start(sin_sb[:], sint[:])
            nc.sync.dma_start(crow_sb[:], crow[:])
            nc.sync.dma_start(mdiag_sb[:], mdiag[:])
            nc.sync.dma_start(mfar_sb[:], mfar[:])
            # ones column of v_aug (softmax-denominator accumulator)
            nc.vector.memset(vaug_sb[:, :, 128], 1.0)

            with tc.tile_pool(name="xt", bufs=3) as xtp, \
                 tc.tile_pool(name="pa", bufs=1, space="PSUM") as pap, \
                 tc.tile_pool(name="pg", bufs=1, space="PSUM") as pgp, \
                 tc.tile_pool(name="ps", bufs=1, space="PSUM") as psp, \
                 tc.tile_pool(name="py", bufs=1, space="PSUM") as pyp, \
                 tc.tile_pool(name="ptq", bufs=1, space="PSUM") as tqp, \
                 tc.tile_pool(name="pj", bufs=2) as pjp, \
                 tc.tile_pool(name="gg", bufs=2) as ggp, \
                 tc.tile_pool(name="qt", bufs=2) as qtp, \
                 tc.tile_pool(name="u", bufs=8) as up, \
                 tc.tile_pool(name="gs", bufs=5) as gsp, \
                 tc.tile_pool(name="os", bufs=3) as osp, \
                 tc.tile_pool(name="aw", bufs=2) as awp, \
                 tc.tile_pool(name="sw", bufs=1) as swp:
                # x-tile DMAs for the first tiles, then the weight stream.
                # DMA transfers serialize, so order = priority: x(0,1),
                # all qkv chunks (gate the whole pipeline), x(2,3) spliced
                # in, then gate chunks (not needed until the first y evac).
                xpre = {}

                def pre_x(ti):
                    xs = xtp.tile([128, 16, 128], f32r, name=f"x_{ti}",
                                  tag="x")
                    nc.sync.dma_start(xs[:], xt[:, ti])
                    xpre[ti] = xs

                pre_x(0)
                pre_x(1)
                for k in range(16):
                    nc.sync.dma_start(wqkv_sb[k][:],
                                      wqkv[k * 128:(k + 1) * 128, :])
                    if k == 7:
                        pre_x(2)
                    if k == 12:
                        pre_x(3)
                for k in range(16):
                    nc.sync.dma_start(wg_sb[k][:], wg[k * 128:(k + 1) * 128, :])
                    if k == 5:
                        pre_x(4)
                    if k == 11:
                        pre_x(5)

                # scores rotate through the 3 banks of s_ps in 2-block
                # passes (WAR is always 3 passes back); y has its own bank
                s_ps = psp.tile([128, 1536], f32, name="s_ps")
                y_ps = pyp.tile([128, 400], f32, name="y_ps")

                def emit_A(ti, qt_tile, half):
                    """Projections + rmsnorm + rope + scales for one
                    128-token tile; q lands transposed in qt_tile[:, :,
                    half*128:], k in kT_sb, v in vaug_sb, gate in gs."""
                    tsl = slice(ti * 128, (ti + 1) * 128)
                    x_sb = xpre.pop(ti, None)
                    if x_sb is None:
                        x_sb = xtp.tile([128, 16, 128], f32r, name=f"x_{ti}",
                                        tag="x")
                        nc.sync.dma_start(x_sb[:], xt[:, ti])
                    pa = pap.tile([128, 768], f32, name=f"pa_{ti}", tag="pa")
                    # fused qkv|krope|g64 projection (tokens on partitions)
                    for k in range(16):
                        nc.tensor.matmul(pa[:, 0:512], x_sb[:, k, :],
                                         wqkv_sb[k][:, 0:512],
                                         start=(k == 0), stop=(k == 15))
                    for k in range(16):
                        nc.tensor.matmul(pa[:, 512:768], x_sb[:, k, :],
                                         wqkv_sb[k][:, 512:768],
                                         start=(k == 0), stop=(k == 15))
                    # evacuate the slab to SBUF in one parallel ACT+DVE
                    # copy; downstream reads SBUF so the single pa buffer
                    # frees after ~0.5us instead of the full scale chain
                    pj = pjp.tile([128, 768], f32, name=f"pj_{ti}", tag="pj")
                    nc.scalar.copy(pj[:, 0:512], pa[:, 0:512])
                    nc.vector.tensor_copy(pj[:, 512:768], pa[:, 512:768])

                    # crow (log-pos * per-head scaler; no rmsnorm input) is
                    # applied FIRST so the rope chain runs in parallel with
                    # the sumsq/ln/exp chain; only the final r-multiply
                    # waits on rmsnorm
                    qsc = awp.tile([128, 4, 128], f32, name=f"qsc_{ti}",
                                   tag="qsc")
                    for h in range(HPC):
                        nc.vector.tensor_scalar(
                            qsc[:, h, :], pj[:, h * 128:(h + 1) * 128],
                            crow_sb[:, ti, h:h + 1], None, OP.mult)

                    # rmsnorm scales: sumsq over each 128-wide head chunk
                    ss = awp.tile([128, 5], f32, name=f"ss_{ti}", tag="ss")
                    sq = awp.tile([128, 128], f32, name=f"sq_{ti}", tag="sq")
                    for hc in range(5):
                        nc.scalar.activation(
                            sq[:], pj[:, hc * 128:(hc + 1) * 128],
                            AF.Square, accum_out=ss[:, hc:hc + 1])
                    # r = rsqrt(mean+eps) via exp(-0.5*ln(.)) -- keeps every
                    # ACT function in the ln/exp table set (no table reloads)
                    lnm = awp.tile([128, 5], f32, name=f"lnm_{ti}", tag="lnm")
                    nc.scalar.activation(lnm[:], ss[:], AF.Ln,
                                         scale=1.0 / 128.0, bias=eps_sb[:])
                    r = awp.tile([128, 5], f32, name=f"r_{ti}", tag="r")
                    nc.scalar.activation(r[:], lnm[:], AF.Exp, scale=-0.5)

                    # q heads: rope batched across all 4 heads via step-0
                    # broadcast of cos/sin (reads crow-scaled copies, so it
                    # does not wait on rmsnorm)
                    cos_t = cos_sb[:, ti, :]
                    sin_t = sin_sb[:, ti, :]
                    cosb = cos_t[:, None, :].broadcast_to([128, 4, 32])
                    sinb = sin_t[:, None, :].broadcast_to([128, 4, 32])
                    x1 = qsc[:, :, 64:96]
                    x2 = qsc[:, :, 96:128]
                    t1 = awp.tile([128, 4, 32], f32, name=f"t1_{ti}", tag="t1")
                    t2 = awp.tile([128, 4, 32], f32, name=f"t2_{ti}", tag="t2")
                    rp = awp.tile([128, 4, 64], f32, name=f"rp_{ti}", tag="rp")
                    nc.vector.tensor_mul(t1[:], x1, cosb)
                    nc.vector.tensor_mul(t2[:], x2, sinb)
                    nc.vector.tensor_add(rp[:, :, 0:32], t1[:], t2[:])
                    nc.vector.tensor_mul(t1[:], x2, cosb)
                    nc.vector.tensor_mul(t2[:], x1, sinb)
                    nc.vector.tensor_sub(rp[:, :, 32:64], t1[:], t2[:])

                    # final rmsnorm scale, then PE-transpose to (d,t)
                    # in two waves through the 1-bank tq pool
                    qf = awp.tile([128, 4, 128], f32r, name=f"qf_{ti}",
                                  tag="qf")
                    for h in range(HPC):
                        nc.vector.tensor_scalar(
                            qf[:, h, 0:64], qsc[:, h, 0:64],
                            r[:, h:h + 1], None, OP.mult)
                        nc.gpsimd.tensor_scalar(
                            qf[:, h, 64:128], rp[:, h, :], r[:, h:h + 1],
                            None, OP.mult)
                    # kv head -> v (token-major) and k tied half
                    nc.vector.tensor_scalar(
                        vaug_sb[:, ti, 0:128], pj[:, 512:640],
                        r[:, 4:5], None, OP.mult)
                    kpre = awp.tile([128, 128], f32r, name=f"kp_{ti}",
                                    tag="kpre")
                    nc.vector.tensor_scalar(
                        kpre[:, 0:64], pj[:, 512:576],
                        r[:, 4:5], None, OP.mult)
                    # k_rope: bias, rope (no norm)
                    krf = awp.tile([128, 64], f32, name=f"krf_{ti}", tag="krf")
                    nc.vector.tensor_add(krf[:], pj[:, 640:704], brk_sb[:])
                    kt1 = awp.tile([128, 32], f32, name=f"kt1_{ti}", tag="t1")
                    kt2 = awp.tile([128, 32], f32, name=f"kt2_{ti}", tag="t2")
                    nc.vector.tensor_mul(kt1[:], krf[:, 0:32], cos_t)
                    nc.vector.tensor_mul(kt2[:], krf[:, 32:64], sin_t)
                    nc.vector.tensor_add(kpre[:, 64:96], kt1[:], kt2[:])
                    nc.vector.tensor_mul(kt1[:], krf[:, 32:64], cos_t)
                    nc.vector.tensor_mul(kt2[:], krf[:, 0:32], sin_t)
                    nc.vector.tensor_sub(kpre[:, 96:128], kt1[:], kt2[:])

                    # transpose wave 1: q heads 0-2
                    tq1 = tqp.tile([128, 384], f32r, name=f"tq1_{ti}",
                                   tag="tq")
                    for h in range(3):
                        nc.tensor.transpose(tq1[:, h * 128:(h + 1) * 128],
                                            qf[:, h, :], ident_sb[:])
                    nc.scalar.copy(
                        qt_tile[:, 0:3, half * 128:half * 128 + 128],
                        tq1[:].rearrange("p (h t) -> p h t", h=3))
                    # transpose wave 2: q head 3 + k
                    tq2 = tqp.tile([128, 256], f32r, name=f"tq2_{ti}",
                                   tag="tq")
                    nc.tensor.transpose(tq2[:, 0:128], qf[:, 3, :],
                                        ident_sb[:])
                    nc.tensor.transpose(tq2[:, 128:256], kpre[:], ident_sb[:])
                    nc.scalar.copy(
                        qt_tile[:, 3, half * 128:half * 128 + 128],
                        tq2[:, 0:128])
                    nc.scalar.copy(kT_sb[:, tsl], tq2[:, 128:256])

                    # gate projection in its own bank, evacuated to SBUF;
                    # the last 64 gate cols rode in the qkv slab at [704:768]
                    pg = pgp.tile([128, 448], f32, name=f"pg_{ti}", tag="pg")
                    for k in range(16):
                        nc.tensor.matmul(pg[:], x_sb[:, k, :],
                                         wg_sb[k][:], start=(k == 0),
                                         stop=(k == 15))
                    gg = ggp.tile([128, 448], f32, name=f"gg_{ti}", tag="gg")
                    nc.vector.tensor_copy(gg[:], pg[:])
                    gs = gsp.tile([128, 512], bf16, name=f"gs_{ti}", tag="gs")
                    for src, dsl in ((gg[:], slice(0, 448)),
                                     (pj[:, 704:768], slice(448, 512))):
                        n = dsl.stop - dsl.start
                        # silu = g / (1 + exp(-g))
                        gsg = swp.tile([128, n], f32, name=f"gsg_{ti}_{n}",
                                       tag="gsg")
                        nc.scalar.activation(gsg[:], src, AF.Exp, scale=-1.0)
                        gw = swp.tile([128, n], f32, name=f"gw_{ti}_{n}",
                                      tag="gw")
                        nc.gpsimd.tensor_scalar_add(gw[:], gsg[:], 1.0)
                        gwi = swp.tile([128, n], f32, name=f"gwi_{ti}_{n}",
                                       tag="gwi")
                        nc.vector.reciprocal(gwi[:], gw[:])
                        nc.vector.tensor_mul(gs[:, dsl], src, gwi[:])
                    return gs

                # ---------------- fused pipeline ----------------
                def emit_Y(t0, t1, tjmin, us, gs0, gs1, os0, os1):
                    for h in range(HPC):
                        u = us[h]
                        # sequential groups through one y region
                        for side, t, gs, osb in ((0, t0, gs0, os0),
                                                 (1, t1, gs1, os1)):
                            wlo = max(0, t - 8) - tjmin
                            whi = t - tjmin
                            nb = whi - wlo + 1
                            for i, w in enumerate(range(wlo, whi + 1)):
                                tj = tjmin + w
                                nc.tensor.matmul(
                                    y_ps[:, 0:129],
                                    u[:, w * 256 + side * 128:
                                      w * 256 + side * 128 + 128],
                                    vaug_sb[:, tj, 0:129],
                                    start=(i == 0), stop=(i == nb - 1))
                            linv = awp.tile([128, 1], f32,
                                            name=f"li_{t}_{h}", tag="li")
                            nc.vector.reciprocal(linv[:], y_ps[:, 128:129])
                            nc.vector.scalar_tensor_tensor(
                                osb[:, h * 128:(h + 1) * 128],
                                y_ps[:, 0:128], linv[:],
                                gs[:, h * 128:(h + 1) * 128],
                                OP.mult, OP.mult)
                    nc.gpsimd.dma_start(out[t0 * 128:(t0 + 1) * 128, :],
                                        os0[:])
                    nc.gpsimd.dma_start(out[t1 * 128:(t1 + 1) * 128, :],
                                        os1[:])

                pend = []
                for pt in range(NPAIR):
                    # Y trails by one pair: the WAR-waiting y groups always
                    # compete against ready score/proj matmuls, not stalls
                    if pend:
                        emit_Y(*pend.pop(0))
                    t0, t1 = 2 * pt, 2 * pt + 1
                    tjmin = max(0, t0 - 8)
                    nblk = t1 - tjmin + 1  # union window blocks (<=10)
                    qt_tile = qtp.tile([128, 4, 256], f32r, name=f"qt_{pt}",
                                       tag="qt")
                    gs0 = emit_A(t0, qt_tile, 0)
                    gs1 = emit_A(t1, qt_tile, 1)

                    os0 = osp.tile([128, 512], f32, name=f"os_{t0}", tag="os")
                    os1 = osp.tile([128, 512], f32, name=f"os_{t1}", tag="os")
                    # masks: diag blocks always; far blocks once the window
                    # is full. corner half-blocks (w0 right of t0-only far
                    # tile, w_last left of t1's diag) are never read by the
                    # y matmuls.
                    wd0 = t0 - tjmin
                    masks = {(wd0, 0): mdiag_sb, (wd0 + 1, 1): mdiag_sb}
                    if t0 >= 8:
                        masks[(0, 0)] = mfar_sb
                        masks[(1, 1)] = mfar_sb
                    wlo1 = max(0, t1 - 8) - tjmin
                    # all four heads' scores + exps first: by the time the
                    # y matmuls run, every exp they need has long finished
                    # (in-order PE never parks on an exp-wait inside y)
                    us = []
                    for h in range(HPC):
                        u = up.tile([128, 2560], bf16, name=f"u_{pt}_{h}",
                                    tag="u")
                        us.append(u)
                        # scores in 2-block passes rotating through the
                        # 3 banks of s_ps: the WAR on a bank is 3 passes
                        # back, so score matmuls never wait on a fresh exp
                        for p, w0 in enumerate(range(0, nblk, 2)):
                            w1 = min(w0 + 2, nblk)
                            bank = (p % 3) * 512
                            for w in range(w0, w1):
                                tj = tjmin + w
                                nc.tensor.matmul(
                                    s_ps[:, bank + (w - w0) * 256:
                                         bank + (w - w0 + 1) * 256],
                                    kT_sb[:, tj * 128:(tj + 1) * 128],
                                    qt_tile[:, h, :],
                                    start=True, stop=True)
                            nc.scalar.activation(
                                u[:, w0 * 256:w1 * 256],
                                s_ps[:, bank:bank + (w1 - w0) * 256], AF.Exp)
                            for w in range(w0, w1):
                                m = masks.get((w, 0))
                                if m is not None:
                                    nc.gpsimd.tensor_mul(
                                        u[:, w * 256:w * 256 + 128],
                                        u[:, w * 256:w * 256 + 128], m[:])
                                m = masks.get((w, 1))
                                if m is not None:
                                    nc.gpsimd.tensor_mul(
                                        u[:, w * 256 + 128:(w + 1) * 256],
                                        u[:, w * 256 + 128:(w + 1) * 256],
                                        m[:])
                    pend.append((t0, t1, tjmin, us, gs0, gs1, os0, os1))

                while pend:
                    emit_Y(*pend.pop(0))

    nc.compile()
    _built["nc"] = nc
    return nc


def _host_inputs(hidden_states, W_qkv, W_rk, b_rk, softmax_scaler, W_g):
    """Per-core input dicts (host-side sharding / constant prep)."""
    inv_freq = 1.0 / (ROPE_BASE ** (np.arange(0, D2, 2, dtype=np.float32) / D2))
    tpos = np.arange(T, dtype=np.float32)
    freqs = tpos[:, None] * inv_freq[None, :]
    cost = np.cos(freqs).astype(np.float32)
    sint = np.sin(freqs).astype(np.float32)
    logpos = np.log(np.minimum(tpos + 1.0, float(WSIZE))).astype(np.float32)
    scale = logpos / np.float32(np.sqrt(D_HEAD))

    ii = np.arange(128)
    mdiag = (ii[:, None] <= ii[None, :]).astype(_BF16)  # (j, i): j <= i
    mfar = (ii[:, None] >= ii[None, :]).astype(_BF16)   # (j, i): j >= i
    ident = np.eye(128, dtype=np.float32)
    brk_t = np.broadcast_to(
        np.asarray(b_rk, np.float32)[None, :], (128, 64)).copy()

    xf = np.asarray(hidden_states, np.float32)
    wqkv_f = np.asarray(W_qkv, np.float32)
    wrk_f = np.asarray(W_rk, np.float32)
    wg_f = np.asarray(W_g, np.float32)
    scaler = np.asarray(softmax_scaler, np.float32)

    # pre-transpose + swizzle x per batch: [128(p), NT, 16(k), 128(t)]
    xtb = []
    for b in range(B):
        xtb.append(np.ascontiguousarray(
            xf[b].reshape(NT, 128, 16, 128).transpose(3, 0, 2, 1)))

    cost_s = np.ascontiguousarray(cost.reshape(NT, 128, 32).transpose(1, 0, 2))
    sint_s = np.ascontiguousarray(sint.reshape(NT, 128, 32).transpose(1, 0, 2))

    in_maps = []
    for c in range(NCORES):
        b, g = c // N_KV, c % N_KV
        qcols = wqkv_f[:, 4 * g * 128:(4 * g + 4) * 128]
        kvcols = wqkv_f[:, (N_HEADS + g) * 128:(N_HEADS + g + 1) * 128]
        gcols = wg_f[:, 4 * g * 128:(4 * g + 4) * 128]
        crow = scale[:, None] * scaler[None, 4 * g:4 * g + 4]
        in_maps.append({
            "xt": xtb[b],
            "wqkv": np.ascontiguousarray(np.concatenate(
                [qcols, kvcols, wrk_f, gcols[:, 448:512]], axis=1)),
            "wg": np.ascontiguousarray(gcols[:, 0:448]),
            "brk": brk_t,
            "cost": cost_s,
            "sint": sint_s,
            "crow": np.ascontiguousarray(
                crow.reshape(NT, 128, HPC).transpose(1, 0, 2)).astype(
                    np.float32),
            "mdiag": mdiag,
            "mfar": mfar,
            "ident": ident,
        })
    return in_maps


def kernel(hidden_states, W_qkv, W_rk, b_rk, softmax_scaler, W_g):
    from concourse.bass_utils import run_bass_kernel_spmd

    nc = _build_nc()
    in_maps = _host_inputs(hidden_states, W_qkv, W_rk, b_rk,
                           softmax_scaler, W_g)
    res = run_bass_kernel_spmd(nc, in_maps, list(range(NCORES)))
    outf = np.empty((B, T, N_HEADS, D_HEAD), np.float32)
    for c in range(NCORES):
        b, g = c // N_KV, c % N_KV
        outf[b, :, 4 * g:4 * g + 4, :] = res.results[c]["out"].reshape(
            T, HPC, D_HEAD)
    return outf
